# revision 3
# baseline (speedup 1.0000x reference)
# Multi-head causal attention for 8 Trainium2 NeuronCores (Bass/Tile).
#
# Problem: q,k,v [2,16,2048,64] f32, bool mask [1,1,2048,2048] (True = masked,
# additive -1e4 bias before softmax in the reference).
#
# Sharding: batch*heads = 32 items, 4 per core (pure data/head parallel, no
# communication).
#
# Per-core kernel (per head), all in "transposed score" layout so softmax'd
# probabilities come out of the ScalarEngine already laid out for the PV
# matmul (keys on partitions):
#   - Q,K arrive pre-transposed from the host ([head, d, seq] fp16), V arrives
#     pre-arranged in its exact SBUF layout [128, nb, 65] with a ones column
#     baked in (so every input DMA is one large contiguous transfer -- the
#     strided V gather used to serialize ~13us of DMA at kernel start).
#   - Per key-block j: S^T_j = K_j Q^T via matmul into PSUM [128, <=1024];
#     exp on ScalarE with the 1/sqrt(64) scale folded in (no row-max
#     subtraction: |scores| <= ~7, exp is safe in f32, and softmax is
#     shift-invariant so the result matches the reference).
#   - Mask handling, decided on the host per 128x128 block from the actual
#     mask input: fully-masked blocks are skipped outright (their probs
#     underflow to exactly 0 in the reference too); mixed blocks multiply
#     the probabilities by a 0/1 keep-tile (equivalent to the -1e4 bias:
#     exp(s - 1e4) == 0 exactly in f32) on the otherwise idle GpSimd engine.
#   - PV accumulates O^T [65, q] in PSUM over key-blocks, with V augmented
#     by a ones-column so row 64 of the accumulator is the softmax
#     denominator for free.
#   - Epilogue (all fp16 staging -- keeps the PE transposes at 1 cyc/col):
#     GpSimd copies the PSUM accumulator halves to SBUF, TensorE transposes
#     O^T back, DVE multiplies by the reciprocal denominator (gathered to
#     [128,16] via a tiny SBUF->SBUF DMA), DMA out.
#   - The PE instruction stream is chained (sync=False deps) in a software-
#     pipelined order so the TensorEngine -- the bottleneck at its throttled
#     sustained rate of ~1 col / 0.83ns -- never waits: QK_j+1 before PV_j,
#     epilogue transposes of head h slotted into head h+1's QK stream.
import numpy as np
from contextlib import ExitStack

B, H, S, D = 2, 16, 2048, 64
NCORES = 8
BH = B * H
HPC = BH // NCORES  # heads per core
BLK = 128
NB = S // BLK  # 16
VW = D + 1  # V columns + ones column
SCALE = 1.0 / 8.0  # 1/sqrt(D)

FREE, SKIP, BIAS = 0, 1, 2

_cache = {}


def _plan_from_mask(mask):
    """Classify 128x128 mask blocks; build unique 0/1 keep-tiles ([key, query]
    orientation) for the mixed blocks."""
    mask2d = np.asarray(mask).reshape(S, S).astype(bool)
    m = mask2d.reshape(NB, BLK, NB, BLK)
    anyb = m.any(axis=(1, 3))
    allb = m.all(axis=(1, 3))
    codes = np.where(allb, SKIP, np.where(anyb, BIAS, FREE)).astype(np.int64)
    # A query row whose whole key range is masked sees a constant bias, which
    # softmax ignores -- the reference then equals unmasked attention. Treat
    # whole such q-blocks as unmasked.
    fq = mask2d.all(axis=1).reshape(NB, BLK).all(axis=1)
    codes[fq, :] = FREE
    tiles = {}
    tile_idx = np.full((NB, NB), -1, dtype=np.int64)
    for qb in range(NB):
        for kb in range(NB):
            if codes[qb, kb] != BIAS:
                continue
            t = np.ascontiguousarray(
                (~mask2d[qb * BLK:(qb + 1) * BLK, kb * BLK:(kb + 1) * BLK].T)
            ).astype(np.float32)
            key = t.tobytes()
            if key not in tiles:
                tiles[key] = (len(tiles), t)
            tile_idx[qb, kb] = tiles[key][0]
    if tiles:
        bt = np.stack([t for _, t in sorted(tiles.values())], axis=0)
    else:
        bt = np.zeros((1, BLK, BLK), np.float32)
    return codes, tile_idx, bt


def _ceil_pieces(c0, c1, step):
    out = []
    c = c0
    while c < c1:
        out.append((c, min(c + step, c1)))
        c = out[-1][1]
    return out


def _runs(blocks):
    """Contiguous runs from a sorted list of block indices."""
    runs = []
    for i in blocks:
        if runs and runs[-1][1] == i:
            runs[-1][1] = i + 1
        else:
            runs.append([i, i + 1])
    return [tuple(r) for r in runs]


def build_nc(codes, tile_idx, n_bt, mmdt_name="float16"):
    import concourse.bass as bass
    import concourse.mybir as mybir
    import concourse.tile as tile
    from concourse import bacc
    from concourse.masks import make_identity
    from concourse.tile_rust import add_dep_helper

    dt = mybir.dt
    f32 = dt.float32
    mmdt = getattr(dt, mmdt_name)
    Exp = mybir.ActivationFunctionType.Exp
    mult = mybir.AluOpType.mult

    # Per key-block: which q-blocks participate.
    active = {j: [i for i in range(NB) if codes[i, j] != SKIP] for j in range(NB)}
    for i in range(NB):
        assert any(codes[i, j] != SKIP for j in range(NB)), (
            "query block with all key blocks masked should be impossible"
        )
    # PV PSUM accumulation start/stop must be managed per 512-column PSUM
    # bank (4 q-blocks): first/last key-block writing each bank.
    NBANK = 4
    bank_first = {}
    bank_last = {}
    for bank in range(NBANK):
        js = [
            j
            for j in range(NB)
            if any(codes[i, j] != SKIP for i in range(bank * 4, bank * 4 + 4))
        ]
        bank_first[bank] = js[0]
        bank_last[bank] = js[-1]

    nc = bacc.Bacc("TRN2", target_bir_lowering=False, debug=False, num_devices=NCORES)
    # Q and K arrive pre-transposed ([head, d, seq]) and V pre-arranged in
    # its SBUF layout ([128, nb*65] with the ones column baked in), all
    # pre-cast to the matmul dtype -- every load is one contiguous DMA.
    qt_d = nc.dram_tensor("qt", [HPC, D, S], mmdt, kind="ExternalInput").ap()
    kt_d = nc.dram_tensor("kt", [HPC, D, S], mmdt, kind="ExternalInput").ap()
    v_d = nc.dram_tensor("v", [HPC, BLK, NB * VW], mmdt, kind="ExternalInput").ap()
    bt_d = nc.dram_tensor("bt", [n_bt, BLK, BLK], mmdt, kind="ExternalInput").ap()
    o_d = nc.dram_tensor("o", [HPC, S, D], f32, kind="ExternalOutput").ap()

    with tile.TileContext(nc) as tc, ExitStack() as ctx:
        const = ctx.enter_context(tc.tile_pool(name="const", bufs=1))
        ldpool = ctx.enter_context(tc.tile_pool(name="ld", bufs=3))
        tpool = ctx.enter_context(tc.tile_pool(name="tp", bufs=3))
        ppool = ctx.enter_context(tc.tile_pool(name="pp", bufs=3))
        otpool = ctx.enter_context(tc.tile_pool(name="ot", bufs=2))
        smpool = ctx.enter_context(tc.tile_pool(name="sm", bufs=3))
        outpool = ctx.enter_context(tc.tile_pool(name="ob", bufs=4))
        scpool = ctx.enter_context(tc.tile_pool(name="sc", bufs=2, space="PSUM"))
        pvpool = ctx.enter_context(tc.tile_pool(name="pv", bufs=2, space="PSUM"))

        identm = const.tile([BLK, BLK], mmdt, tag="identm")
        make_identity(nc, identm[:])
        bts = []
        for u in range(n_bt):
            t = const.tile([BLK, BLK], mmdt, tag=f"bt{u}", name=f"bt_sb{u}")
            nc.sync.dma_start(out=t[:], in_=bt_d[u])
            bts.append(t)

        # PE-order bookkeeping: weight reloads cost ~330ns per stationary
        # swap, so same-weight matmuls must run adjacently. We collect the
        # PE instructions and chain them (sync=False deps) in a software-
        # pipelined order: QK_j+1 before PV_j, epilogue transposes of head h
        # slotted early into head h+1's stream.
        qk_h = []
        pv_h = []
        epi_h = []
        epi0_h = []
        for h in range(HPC):
            qk_groups = []
            pv_groups = []
            epi_insts = []
            epi0_insts = []
            # ---- load this head's Q^T, K^T, V (one contiguous DMA each) ----
            qt = tpool.tile([D, S], mmdt, tag="qt")
            kt = tpool.tile([D, S], mmdt, tag="kt")
            nc.sync.dma_start(out=qt[:], in_=qt_d[h])
            nc.sync.dma_start(out=kt[:], in_=kt_d[h])
            vno = ldpool.tile([BLK, NB * VW], mmdt, tag="vn")
            nc.sync.dma_start(out=vno[:], in_=v_d[h])
            vno3 = vno[:].rearrange("p (n c) -> p n c", c=VW)

            # ---- main loop over key blocks ----
            pvh = [
                pvpool.tile([VW, 1024], f32, tag="pv", name=f"pv{h}_{i}")
                for i in range(2)
            ]
            for j in range(NB):
                blocks = active[j]
                if not blocks:
                    continue
                pT = ppool.tile([BLK, S], mmdt, tag="pT")
                qk_g = []
                for (r0, r1) in _runs(blocks):
                    for (c0, c1) in _ceil_pieces(r0 * BLK, r1 * BLK, 1024):
                        w = c1 - c0
                        sc = scpool.tile([BLK, w], f32, tag="sc")
                        for (s0, s1) in _ceil_pieces(0, w, 512):
                            qk_g.append(nc.tensor.matmul(
                                sc[:, s0:s1],
                                lhsT=kt[:, j * BLK:(j + 1) * BLK],
                                rhs=qt[:, c0 + s0:c0 + s1],
                                start=True,
                                stop=True,
                            ))
                        nc.scalar.activation(pT[:, c0:c1], sc[:, 0:w], Exp, scale=SCALE)
                qk_groups.append(qk_g)
                pv_g = []
                # mixed blocks: zero the masked probabilities (gpsimd is idle)
                for i in blocks:
                    if codes[i, j] == BIAS:
                        sl = pT[:, i * BLK:(i + 1) * BLK]
                        nc.gpsimd.tensor_tensor(sl, sl, bts[tile_idx[i, j]][:], mult)
                # PV accumulation: start/stop flags at PSUM-bank granularity
                bank_order = sorted(
                    range(NBANK),
                    key=lambda b: any(
                        codes[i, j] == BIAS
                        for i in range(b * 4, b * 4 + 4)
                        if i in blocks
                    ),
                )
                for bank in bank_order:
                    bi = [i for i in blocks if bank * 4 <= i < bank * 4 + 4]
                    if not bi:
                        continue
                    half = bank // 2
                    toff = half * 1024  # tile-relative offset of this half
                    is_last = j == bank_last[bank]
                    if j == bank_first[bank]:
                        # first write: one full-bank matmul so every column
                        # starts with start=True; zero any inactive columns
                        # of pT first (no-op for causal/empty masks).
                        for i in range(bank * 4, bank * 4 + 4):
                            if i not in bi:
                                nc.gpsimd.memset(
                                    pT[:, i * BLK:(i + 1) * BLK], 0.0
                                )
                        g0, g1 = bank * 4 * BLK, (bank + 1) * 4 * BLK
                        pv_g.append(nc.tensor.matmul(
                            pvh[half][:, g0 - toff:g1 - toff],
                            lhsT=vno3[:, j, :],
                            rhs=pT[:, g0:g1],
                            start=True,
                            stop=is_last,
                        ))
                    else:
                        runs = _runs(bi)
                        for ri, (r0, r1) in enumerate(runs):
                            pv_g.append(nc.tensor.matmul(
                                pvh[half][:, r0 * BLK - toff:r1 * BLK - toff],
                                lhsT=vno3[:, j, :],
                                rhs=pT[:, r0 * BLK:r1 * BLK],
                                start=False,
                                stop=is_last and ri == len(runs) - 1,
                            ))
                pv_groups.append(pv_g)

            # ---- epilogue: normalize and write out ----
            # Half 0 of the PV accumulator is complete once its last key
            # block is done (j=bank_last[1]), so its copy + retranspose are
            # scheduled mid-k-loop; half 1 and the denominators drain into
            # the next head's stream. All staging is fp16: the PE transposes
            # run at 1 cyc/col (vs 2 for f32) and SBUF/PSUM traffic halves.
            ot = otpool.tile([VW, S], mmdt, tag="ot")
            for half in range(2):
                # (GpSimd can't read PSUM -- neuronxcc rejects the lowering)
                nc.vector.tensor_copy(
                    ot[:, half * 1024:(half + 1) * 1024], pvh[half][:, :]
                )
            # half-0 output transpose first: takes the PSUM slot freed by
            # pvh[0] (mid-loop), before dnt claims pvh[1]'s slot.
            rts = []
            for half in range(2):
                rt = pvpool.tile([BLK, 512], mmdt, tag="pv", name=f"rt{h}_{half}")
                dst_list = epi0_insts if half == 0 else epi_insts
                for u in range(8):
                    i = half * 8 + u
                    dst_list.append(nc.tensor.transpose(
                        rt[:, u * D:(u + 1) * D],
                        ot[0:D, i * BLK:(i + 1) * BLK],
                        identm[0:D, 0:D],
                    ))
                rts.append(rt)
                if half == 0:
                    # gather denominators [1, S] -> [16, 128] -> [128, 16]
                    dq = smpool.tile([NB, BLK], mmdt, tag="dq")
                    nc.sync.dma_start(out=dq[:], in_=ot[D:VW, :])
                    dntp = pvpool.tile([BLK, NB], mmdt, tag="pv", name=f"dnt{h}")
                    epi_insts.append(
                        nc.tensor.transpose(dntp[:], dq[:], identm[0:NB, 0:NB])
                    )
                    rcp = smpool.tile([BLK, NB], f32, tag="rcp")
                    nc.vector.reciprocal(rcp[:], dntp[:])
            for half in range(2):
                osb = outpool.tile([BLK, 512], f32, tag="ob")
                nc.vector.tensor_tensor(
                    osb[:].rearrange("p (u d) -> p u d", d=D),
                    rts[half][:].rearrange("p (u d) -> p u d", d=D),
                    rcp[:, half * 8:(half + 1) * 8]
                    .rearrange("p (u o) -> p u o", o=1)
                    .broadcast_to([BLK, 8, D]),
                    mult,
                )
                nc.sync.dma_start(
                    out=o_d[h].rearrange("(n p) d -> p n d", p=BLK)[
                        :, half * 8:(half + 1) * 8, :
                    ],
                    in_=osb[:].rearrange("p (u d) -> p u d", d=D),
                )
            qk_h.append(qk_groups)
            pv_h.append(pv_groups)
            epi_h.append(epi_insts)
            epi0_h.append(epi0_insts)

        # Build the PE ordering chain.
        chain = []
        safe0 = max(bank_last[0], bank_last[1]) + 2
        for h in range(HPC):
            qk = qk_h[h]
            pv = pv_h[h]
            assert len(qk) == len(pv)
            placed0 = False
            if qk:
                chain += qk[0]
            for idx in range(1, len(qk)):
                chain += qk[idx]
                if idx == 1 and h > 0:
                    # prev head's epilogue after two QK groups (must still
                    # precede pv[0], which needs the PSUM slots it frees)
                    chain += epi_h[h - 1]
                if idx == safe0 and not placed0:
                    chain += epi0_h[h]
                    placed0 = True
                chain += pv[idx - 1]
            if pv:
                chain += pv[-1]
            if not placed0:
                chain += epi0_h[h]
        chain += epi_h[HPC - 1]
        for a, b in zip(chain, chain[1:]):
            add_dep_helper(b.ins, a.ins, sync=False, reason="pe weight-group order")
    nc.compile()
    return nc


MM_DT = __import__("os").environ.get("ATTN_MM_DT", "float16")


def _get_program(mask):
    codes, tile_idx, bt = _plan_from_mask(mask)
    key = (codes.tobytes(), tile_idx.tobytes(), bt.tobytes(), MM_DT)
    if key not in _cache:
        _cache[key] = (build_nc(codes, tile_idx, bt.shape[0], MM_DT), bt)
    return _cache[key]


LAST_RESULTS = None  # BassKernelResults of the most recent run (for profiling)


def kernel(q, k, v, mask):
    global LAST_RESULTS
    from concourse.bass_utils import run_bass_kernel_spmd
    import ml_dtypes

    npdt = {"float16": np.float16, "bfloat16": ml_dtypes.bfloat16}[MM_DT]
    nc, bt = _get_program(mask)
    qf = np.asarray(q, np.float32).reshape(BH, S, D)
    kf = np.asarray(k, np.float32).reshape(BH, S, D)
    vf = np.asarray(v, np.float32).reshape(BH, S, D)
    # V in its SBUF layout: [128, nb, 65] per head, ones column baked in.
    vr = vf.reshape(BH, NB, BLK, D).transpose(0, 2, 1, 3)  # [BH, 128, NB, D]
    vno = np.concatenate(
        [vr, np.ones((BH, BLK, NB, 1), np.float32)], axis=3
    ).reshape(BH, BLK, NB * VW).astype(npdt)
    bt16 = bt.astype(npdt)
    in_maps = []
    for c in range(NCORES):
        sl = slice(c * HPC, (c + 1) * HPC)
        in_maps.append({
            # per-shard layout: Q/K shipped [head, d, seq], pre-cast
            "qt": np.ascontiguousarray(qf[sl].transpose(0, 2, 1)).astype(npdt),
            "kt": np.ascontiguousarray(kf[sl].transpose(0, 2, 1)).astype(npdt),
            "v": vno[sl],
            "bt": bt16,
        })
    res = run_bass_kernel_spmd(nc, in_maps, list(range(NCORES)))
    LAST_RESULTS = res
    out = np.concatenate([res.results[c]["o"] for c in range(NCORES)], axis=0)
    return out.reshape(B, H, S, D).astype(np.float32)


# revision 12
# speedup vs baseline: 1.1697x; 1.1697x over previous
# Multi-head causal attention for 8 Trainium2 NeuronCores (Bass/Tile).
#
# Problem: q,k,v [2,16,2048,64] f32, bool mask [1,1,2048,2048] (True = masked,
# additive -1e4 bias before softmax in the reference).
#
# Sharding: batch*heads = 32 items, 4 per core (pure data/head parallel, no
# communication).
#
# Per-core kernel (per head), all in "transposed score" layout so softmax'd
# probabilities come out of the ScalarEngine already laid out for the PV
# matmul (keys on partitions):
#   - Q,K arrive pre-transposed from the host ([head, d, seq] fp16), V arrives
#     pre-arranged in its exact SBUF layout [128, nb, 65] with a ones column
#     baked in (so every input DMA is one large contiguous transfer -- the
#     strided V gather used to serialize ~13us of DMA at kernel start).
#   - Per key-block j: S^T_j = K_j Q^T via matmul into PSUM [128, <=1024];
#     exp on ScalarE with the 1/sqrt(64) scale folded in (no row-max
#     subtraction: |scores| <= ~7, exp is safe in f32, and softmax is
#     shift-invariant so the result matches the reference).
#   - Mask handling, decided on the host per 128x128 block from the actual
#     mask input: fully-masked blocks are skipped outright (their probs
#     underflow to exactly 0 in the reference too); mixed blocks multiply
#     the probabilities by a 0/1 keep-tile (equivalent to the -1e4 bias:
#     exp(s - 1e4) == 0 exactly in f32) on the otherwise idle GpSimd engine.
#   - PV accumulates O^T [65, q] in PSUM over key-blocks, with V augmented
#     by a ones-column so row 64 of the accumulator is the softmax
#     denominator for free.
#   - Epilogue (all fp16 staging -- keeps the PE transposes at 1 cyc/col):
#     GpSimd copies the PSUM accumulator halves to SBUF, TensorE transposes
#     O^T back, DVE multiplies by the reciprocal denominator (gathered to
#     [128,16] via a tiny SBUF->SBUF DMA), DMA out.
#   - The PE instruction stream is chained (sync=False deps) in a software-
#     pipelined order so the TensorEngine -- the bottleneck at its throttled
#     sustained rate of ~1 col / 0.83ns -- never waits: QK_j+1 before PV_j,
#     epilogue transposes of head h slotted into head h+1's QK stream.
import numpy as np
from contextlib import ExitStack

B, H, S, D = 2, 16, 2048, 64
NCORES = 8
BH = B * H
HPC = BH // NCORES  # heads per core
BLK = 128
NB = S // BLK  # 16
VW = D + 1  # V columns + ones column
SCALE = 1.0 / 8.0  # 1/sqrt(D)

FREE, SKIP, BIAS = 0, 1, 2

_cache = {}


def _plan_from_mask(mask):
    """Classify 128x128 mask blocks; build unique 0/1 keep-tiles ([key, query]
    orientation) for the mixed blocks."""
    mask2d = np.asarray(mask).reshape(S, S).astype(bool)
    m = mask2d.reshape(NB, BLK, NB, BLK)
    anyb = m.any(axis=(1, 3))
    allb = m.all(axis=(1, 3))
    codes = np.where(allb, SKIP, np.where(anyb, BIAS, FREE)).astype(np.int64)
    # A query row whose whole key range is masked sees a constant bias, which
    # softmax ignores -- the reference then equals unmasked attention. Treat
    # whole such q-blocks as unmasked.
    fq = mask2d.all(axis=1).reshape(NB, BLK).all(axis=1)
    codes[fq, :] = FREE
    tiles = {}
    tile_idx = np.full((NB, NB), -1, dtype=np.int64)
    for qb in range(NB):
        for kb in range(NB):
            if codes[qb, kb] != BIAS:
                continue
            t = np.ascontiguousarray(
                (~mask2d[qb * BLK:(qb + 1) * BLK, kb * BLK:(kb + 1) * BLK].T)
            ).astype(np.float32)
            key = t.tobytes()
            if key not in tiles:
                tiles[key] = (len(tiles), t)
            tile_idx[qb, kb] = tiles[key][0]
    if tiles:
        bt = np.stack([t for _, t in sorted(tiles.values())], axis=0)
    else:
        bt = np.zeros((1, BLK, BLK), np.float32)
    return codes, tile_idx, bt


def _ceil_pieces(c0, c1, step):
    out = []
    c = c0
    while c < c1:
        out.append((c, min(c + step, c1)))
        c = out[-1][1]
    return out


def _runs(blocks):
    """Contiguous runs from a sorted list of block indices."""
    runs = []
    for i in blocks:
        if runs and runs[-1][1] == i:
            runs[-1][1] = i + 1
        else:
            runs.append([i, i + 1])
    return [tuple(r) for r in runs]


def build_nc(codes, tile_idx, n_bt, mmdt_name="float16"):
    import concourse.bass as bass
    import concourse.mybir as mybir
    import concourse.tile as tile
    from concourse import bacc
    from concourse.masks import make_identity
    from concourse.tile_rust import add_dep_helper

    dt = mybir.dt
    f32 = dt.float32
    mmdt = getattr(dt, mmdt_name)
    Exp = mybir.ActivationFunctionType.Exp
    mult = mybir.AluOpType.mult

    # Per key-block: which q-blocks participate.
    active = {j: [i for i in range(NB) if codes[i, j] != SKIP] for j in range(NB)}
    for i in range(NB):
        assert any(codes[i, j] != SKIP for j in range(NB)), (
            "query block with all key blocks masked should be impossible"
        )
    # PV PSUM accumulation start/stop must be managed per 512-column PSUM
    # bank (4 q-blocks): first/last key-block writing each bank.
    NBANK = 4
    bank_first = {}
    bank_last = {}
    for bank in range(NBANK):
        js = [
            j
            for j in range(NB)
            if any(codes[i, j] != SKIP for i in range(bank * 4, bank * 4 + 4))
        ]
        bank_first[bank] = js[0]
        bank_last[bank] = js[-1]

    nc = bacc.Bacc("TRN2", target_bir_lowering=False, debug=False, num_devices=NCORES)
    # Q and K arrive pre-transposed ([head, d, seq]) and V pre-arranged in
    # its SBUF layout ([128, nb*65] with the ones column baked in), all
    # pre-cast to the matmul dtype -- every load is one contiguous DMA.
    qt_d = nc.dram_tensor("qt", [HPC, D, S], mmdt, kind="ExternalInput").ap()
    kt_d = nc.dram_tensor("kt", [HPC, D, S], mmdt, kind="ExternalInput").ap()
    v_d = nc.dram_tensor("v", [HPC, BLK, NB * VW], mmdt, kind="ExternalInput").ap()
    bt_d = nc.dram_tensor("bt", [n_bt, BLK, BLK], mmdt, kind="ExternalInput").ap()
    o_d = nc.dram_tensor("o", [HPC, S, D], mmdt, kind="ExternalOutput").ap()

    with tile.TileContext(nc) as tc, ExitStack() as ctx:
        const = ctx.enter_context(tc.tile_pool(name="const", bufs=1))
        ldpool = ctx.enter_context(tc.tile_pool(name="ld", bufs=HPC))
        tpool = ctx.enter_context(tc.tile_pool(name="tp", bufs=HPC))
        ppool = ctx.enter_context(tc.tile_pool(name="pp", bufs=3))
        otpool = ctx.enter_context(tc.tile_pool(name="ot", bufs=2))
        smpool = ctx.enter_context(tc.tile_pool(name="sm", bufs=3))
        outpool = ctx.enter_context(tc.tile_pool(name="ob", bufs=4))
        scpool = ctx.enter_context(tc.tile_pool(name="sc", bufs=2, space="PSUM"))
        pvpool = ctx.enter_context(tc.tile_pool(name="pv", bufs=2, space="PSUM"))

        identm = const.tile([BLK, BLK], mmdt, tag="identm")
        make_identity(nc, identm[:])
        bts = []
        for u in range(n_bt):
            t = const.tile([BLK, BLK], mmdt, tag=f"bt{u}", name=f"bt_sb{u}")
            nc.sync.dma_start(out=t[:], in_=bt_d[u])
            bts.append(t)

        # PE-order bookkeeping: weight reloads cost ~330ns per stationary
        # swap, so same-weight matmuls must run adjacently. We collect the
        # PE instructions and chain them (sync=False deps) in a software-
        # pipelined order: QK_j+1 before PV_j, epilogue transposes of head h
        # slotted early into head h+1's stream.
        # ---- input loads for all heads, hoisted ahead of the compute so no
        # load DMA trigger ever sits behind a blocking compute instruction on
        # its engine queue. The DMA hardware round-robins active transfers,
        # so a transfer's latency scales with how many are in flight: head
        # 0's loads are split fine (k block 0 and the first q piece finish
        # fast, letting the PE start ~4us earlier) and later heads' loads are
        # gated on head 0's first matmul (via add_dep below) so they don't
        # steal preamble bandwidth.
        qts, kts, vnos, load_insts = [], [], [], []
        for h in range(HPC):
            lds = []
            qt = tpool.tile([D, S], mmdt, tag="qt", name=f"qt{h}")
            kt = tpool.tile([D, S], mmdt, tag="kt", name=f"kt{h}")
            lds.append(nc.scalar.dma_start(out=kt[:, 0:BLK], in_=kt_d[h][:, 0:BLK]))
            if h == 0:
                for (c0, c1) in _ceil_pieces(0, S, 512):
                    lds.append(nc.sync.dma_start(
                        out=qt[:, c0:c1], in_=qt_d[h][:, c0:c1]
                    ))
            else:
                lds.append(nc.sync.dma_start(out=qt[:], in_=qt_d[h]))
            lds.append(nc.scalar.dma_start(out=kt[:, BLK:], in_=kt_d[h][:, BLK:]))
            vno = ldpool.tile([BLK, NB * VW], mmdt, tag="vn", name=f"vn{h}")
            lds.append(nc.gpsimd.dma_start(out=vno[:], in_=v_d[h]))
            qts.append(qt)
            kts.append(kt)
            vnos.append(vno)
            load_insts.append(lds)

        qk_h = []
        pv_h = []
        epi_h = []
        epi0_h = []
        for h in range(HPC):
            qk_groups = []
            pv_groups = []
            epi_insts = []
            epi0_insts = []
            qt, kt, vno = qts[h], kts[h], vnos[h]
            vno3 = vno[:].rearrange("p (n c) -> p n c", c=VW)

            # ---- main loop over key blocks ----
            pvh = [
                pvpool.tile([VW, 1024], f32, tag="pv", name=f"pv{h}_{i}")
                for i in range(2)
            ]
            for j in range(NB):
                blocks = active[j]
                if not blocks:
                    continue
                pT = ppool.tile([BLK, S], mmdt, tag="pT")
                qk_g = []
                for (r0, r1) in _runs(blocks):
                    for (c0, c1) in _ceil_pieces(r0 * BLK, r1 * BLK, 1024):
                        w = c1 - c0
                        sc = scpool.tile([BLK, w], f32, tag="sc")
                        for (s0, s1) in _ceil_pieces(0, w, 512):
                            qk_g.append(nc.tensor.matmul(
                                sc[:, s0:s1],
                                lhsT=kt[:, j * BLK:(j + 1) * BLK],
                                rhs=qt[:, c0 + s0:c0 + s1],
                                start=True,
                                stop=True,
                            ))
                        nc.scalar.activation(pT[:, c0:c1], sc[:, 0:w], Exp, scale=SCALE)
                qk_groups.append(qk_g)
                pv_g = []
                # mixed blocks: zero the masked probabilities (gpsimd is idle)
                for i in blocks:
                    if codes[i, j] == BIAS:
                        sl = pT[:, i * BLK:(i + 1) * BLK]
                        nc.gpsimd.tensor_tensor(sl, sl, bts[tile_idx[i, j]][:], mult)
                # PV accumulation: start/stop flags at PSUM-bank granularity
                bank_order = sorted(
                    range(NBANK),
                    key=lambda b: any(
                        codes[i, j] == BIAS
                        for i in range(b * 4, b * 4 + 4)
                        if i in blocks
                    ),
                )
                for bank in bank_order:
                    bi = [i for i in blocks if bank * 4 <= i < bank * 4 + 4]
                    if not bi:
                        continue
                    half = bank // 2
                    toff = half * 1024  # tile-relative offset of this half
                    is_last = j == bank_last[bank]
                    if j == bank_first[bank]:
                        # first write: one full-bank matmul so every column
                        # starts with start=True; zero any inactive columns
                        # of pT first (no-op for causal/empty masks).
                        for i in range(bank * 4, bank * 4 + 4):
                            if i not in bi:
                                nc.gpsimd.memset(
                                    pT[:, i * BLK:(i + 1) * BLK], 0.0
                                )
                        g0, g1 = bank * 4 * BLK, (bank + 1) * 4 * BLK
                        pv_g.append(nc.tensor.matmul(
                            pvh[half][:, g0 - toff:g1 - toff],
                            lhsT=vno3[:, j, :],
                            rhs=pT[:, g0:g1],
                            start=True,
                            stop=is_last,
                        ))
                    else:
                        runs = _runs(bi)
                        for ri, (r0, r1) in enumerate(runs):
                            pv_g.append(nc.tensor.matmul(
                                pvh[half][:, r0 * BLK - toff:r1 * BLK - toff],
                                lhsT=vno3[:, j, :],
                                rhs=pT[:, r0 * BLK:r1 * BLK],
                                start=False,
                                stop=is_last and ri == len(runs) - 1,
                            ))
                pv_groups.append(pv_g)

            # ---- epilogue: normalize and write out ----
            # Half 0 of the PV accumulator is complete once its last key
            # block is done (j=bank_last[1]), so its copy + retranspose are
            # scheduled mid-k-loop; half 1 and the denominators drain into
            # the next head's stream. All staging is fp16: the PE transposes
            # run at 1 cyc/col (vs 2 for f32) and SBUF/PSUM traffic halves.
            ot = otpool.tile([VW, S], mmdt, tag="ot")
            # Each half: drain the PSUM accumulator to SBUF (512-wide pieces
            # so the retransposes can chase the drain; GpSimd can't read PSUM
            # -- neuronxcc rejects that lowering), retranspose the 8 blocks
            # into rt (cols 0:512) and the denominator row into rt cols
            # 512:520 (sharing the PSUM tile keeps the pvpool ring simple),
            # then normalize + DMA out. Half 0 completes mid-k-loop (its last
            # key block is j=7) so its drain overlaps the tail of the loop;
            # half 1 drains into the next head's stream. Per-half emission
            # order matters: the DVE executes its queue in order, so half 0's
            # reciprocal must not sit behind half 1's (j=15-gated) drain.
            for half in range(2):
                for piece in range(2):
                    nc.vector.tensor_copy(
                        ot[:, half * 1024 + piece * 512:half * 1024 + piece * 512 + 512],
                        pvh[half][:, piece * 512:piece * 512 + 512],
                    )
                rt = pvpool.tile([BLK, 520], mmdt, tag="pv", name=f"rt{h}_{half}")
                dst_list = epi0_insts if half == 0 else epi_insts
                for u in range(8):
                    i = half * 8 + u
                    dst_list.append(nc.tensor.transpose(
                        rt[:, u * D:(u + 1) * D],
                        ot[0:D, i * BLK:(i + 1) * BLK],
                        identm[0:D, 0:D],
                    ))
                # gather denominators [1, 1024] -> [8, 128] -> [128, 8]
                dq = smpool.tile([8, BLK], mmdt, tag="dq")
                nc.sync.dma_start(
                    out=dq[:], in_=ot[D:VW, half * 1024:(half + 1) * 1024]
                )
                dst_list.append(nc.tensor.transpose(
                    rt[:, 512:520], dq[:], identm[0:8, 0:8]
                ))
                rcp = smpool.tile([BLK, 8], f32, tag="rcp")
                nc.vector.reciprocal(rcp[:], rt[:, 512:520])
                osb = outpool.tile([BLK, 512], mmdt, tag="ob")
                nc.vector.tensor_tensor(
                    osb[:].rearrange("p (u d) -> p u d", d=D),
                    rt[:, 0:512].rearrange("p (u d) -> p u d", d=D),
                    rcp[:]
                    .rearrange("p (u o) -> p u o", o=1)
                    .broadcast_to([BLK, 8, D]),
                    mult,
                )
                nc.sync.dma_start(
                    out=o_d[h].rearrange("(n p) d -> p n d", p=BLK)[
                        :, half * 8:(half + 1) * 8, :
                    ],
                    in_=osb[:].rearrange("p (u d) -> p u d", d=D),
                )
            qk_h.append(qk_groups)
            pv_h.append(pv_groups)
            epi_h.append(epi_insts)
            epi0_h.append(epi0_insts)

        # Later heads' input loads wait for head 0's first matmul: the DMA
        # engines round-robin active transfers, so an ungated 1.8MB preload
        # burst would delay the first K/Q tiles (and the first matmul) by
        # several microseconds.
        first_mm = qk_h[0][0][0]
        for h in range(1, HPC):
            for ld in load_insts[h]:
                add_dep_helper(ld.ins, first_mm.ins, sync=True, reason="preload gate")

        # Build the PE ordering chain.
        chain = []
        safe0 = max(bank_last[0], bank_last[1]) + 2
        for h in range(HPC):
            qk = qk_h[h]
            pv = pv_h[h]
            assert len(qk) == len(pv)
            placed0 = False
            if qk:
                chain += qk[0]
            for idx in range(1, len(qk)):
                chain += qk[idx]
                if idx == 1 and h > 0:
                    # prev head's epilogue after two QK groups (must still
                    # precede pv[0], which needs the PSUM slots it frees)
                    chain += epi_h[h - 1]
                if idx == safe0 and not placed0:
                    chain += epi0_h[h]
                    placed0 = True
                chain += pv[idx - 1]
            if pv:
                chain += pv[-1]
            if not placed0:
                chain += epi0_h[h]
        chain += epi_h[HPC - 1]
        for a, b in zip(chain, chain[1:]):
            add_dep_helper(b.ins, a.ins, sync=False, reason="pe weight-group order")
    nc.compile()
    return nc


MM_DT = __import__("os").environ.get("ATTN_MM_DT", "float16")


def _get_program(mask):
    codes, tile_idx, bt = _plan_from_mask(mask)
    key = (codes.tobytes(), tile_idx.tobytes(), bt.tobytes(), MM_DT)
    if key not in _cache:
        _cache[key] = (build_nc(codes, tile_idx, bt.shape[0], MM_DT), bt)
    return _cache[key]


LAST_RESULTS = None  # BassKernelResults of the most recent run (for profiling)


def kernel(q, k, v, mask):
    global LAST_RESULTS
    from concourse.bass_utils import run_bass_kernel_spmd
    import ml_dtypes

    npdt = {"float16": np.float16, "bfloat16": ml_dtypes.bfloat16}[MM_DT]
    nc, bt = _get_program(mask)
    qf = np.asarray(q, np.float32).reshape(BH, S, D)
    kf = np.asarray(k, np.float32).reshape(BH, S, D)
    vf = np.asarray(v, np.float32).reshape(BH, S, D)
    # V in its SBUF layout: [128, nb, 65] per head, ones column baked in.
    vr = vf.reshape(BH, NB, BLK, D).transpose(0, 2, 1, 3)  # [BH, 128, NB, D]
    vno = np.concatenate(
        [vr, np.ones((BH, BLK, NB, 1), np.float32)], axis=3
    ).reshape(BH, BLK, NB * VW).astype(npdt)
    bt16 = bt.astype(npdt)
    in_maps = []
    for c in range(NCORES):
        sl = slice(c * HPC, (c + 1) * HPC)
        in_maps.append({
            # per-shard layout: Q/K shipped [head, d, seq], pre-cast
            "qt": np.ascontiguousarray(qf[sl].transpose(0, 2, 1)).astype(npdt),
            "kt": np.ascontiguousarray(kf[sl].transpose(0, 2, 1)).astype(npdt),
            "v": vno[sl],
            "bt": bt16,
        })
    res = run_bass_kernel_spmd(nc, in_maps, list(range(NCORES)))
    LAST_RESULTS = res
    out = np.concatenate(
        [np.asarray(res.results[c]["o"]) for c in range(NCORES)], axis=0
    )
    return out.reshape(B, H, S, D).astype(np.float32)


# revision 20
# speedup vs baseline: 1.1781x; 1.0072x over previous
# Multi-head causal attention for 8 Trainium2 NeuronCores (Bass/Tile).
#
# Problem: q,k,v [2,16,2048,64] f32, bool mask [1,1,2048,2048] (True = masked,
# additive -1e4 bias before softmax in the reference).
#
# Sharding: batch*heads = 32 items, 4 per core (pure data/head parallel, no
# communication).
#
# Per-core kernel (per head), all in "transposed score" layout so softmax'd
# probabilities come out of the ScalarEngine already laid out for the PV
# matmul (keys on partitions):
#   - Q,K arrive pre-transposed from the host ([head, d, seq] fp16), V arrives
#     pre-arranged in its exact SBUF layout [128, nb, 65] with a ones column
#     baked in (so every input DMA is one large contiguous transfer -- the
#     strided V gather used to serialize ~13us of DMA at kernel start).
#   - Per key-block j: S^T_j = K_j Q^T via matmul into PSUM [128, <=1024];
#     exp on ScalarE with the 1/sqrt(64) scale folded in (no row-max
#     subtraction: |scores| <= ~7, exp is safe in f32, and softmax is
#     shift-invariant so the result matches the reference).
#   - Mask handling, decided on the host per 128x128 block from the actual
#     mask input: fully-masked blocks are skipped outright (their probs
#     underflow to exactly 0 in the reference too); mixed blocks multiply
#     the probabilities by a 0/1 keep-tile (equivalent to the -1e4 bias:
#     exp(s - 1e4) == 0 exactly in f32) on the otherwise idle GpSimd engine.
#   - PV accumulates O^T [65, q] in PSUM over key-blocks, with V augmented
#     by a ones-column so row 64 of the accumulator is the softmax
#     denominator for free.
#   - Epilogue (all fp16 staging -- keeps the PE transposes at 1 cyc/col):
#     GpSimd copies the PSUM accumulator halves to SBUF, TensorE transposes
#     O^T back, DVE multiplies by the reciprocal denominator (gathered to
#     [128,16] via a tiny SBUF->SBUF DMA), DMA out.
#   - The PE instruction stream is chained (sync=False deps) in a software-
#     pipelined order so the TensorEngine -- the bottleneck at its throttled
#     sustained rate of ~1 col / 0.83ns -- never waits: QK_j+1 before PV_j,
#     epilogue transposes of head h slotted into head h+1's QK stream.
import numpy as np
from contextlib import ExitStack

B, H, S, D = 2, 16, 2048, 64
NCORES = 8
BH = B * H
HPC = BH // NCORES  # heads per core
BLK = 128
NB = S // BLK  # 16
VW = D + 1  # V columns + ones column
SCALE = 1.0 / 8.0  # 1/sqrt(D)

FREE, SKIP, BIAS = 0, 1, 2

_cache = {}


def _plan_from_mask(mask):
    """Classify 128x128 mask blocks; build unique 0/1 keep-tiles ([key, query]
    orientation) for the mixed blocks."""
    mask2d = np.asarray(mask).reshape(S, S).astype(bool)
    m = mask2d.reshape(NB, BLK, NB, BLK)
    anyb = m.any(axis=(1, 3))
    allb = m.all(axis=(1, 3))
    codes = np.where(allb, SKIP, np.where(anyb, BIAS, FREE)).astype(np.int64)
    # A query row whose whole key range is masked sees a constant bias, which
    # softmax ignores -- the reference then equals unmasked attention. Treat
    # whole such q-blocks as unmasked.
    fq = mask2d.all(axis=1).reshape(NB, BLK).all(axis=1)
    codes[fq, :] = FREE
    tiles = {}
    tile_idx = np.full((NB, NB), -1, dtype=np.int64)
    for qb in range(NB):
        for kb in range(NB):
            if codes[qb, kb] != BIAS:
                continue
            t = np.ascontiguousarray(
                (~mask2d[qb * BLK:(qb + 1) * BLK, kb * BLK:(kb + 1) * BLK].T)
            ).astype(np.float32)
            key = t.tobytes()
            if key not in tiles:
                tiles[key] = (len(tiles), t)
            tile_idx[qb, kb] = tiles[key][0]
    if tiles:
        bt = np.stack([t for _, t in sorted(tiles.values())], axis=0)
    else:
        bt = np.zeros((1, BLK, BLK), np.float32)
    return codes, tile_idx, bt


def _ceil_pieces(c0, c1, step):
    out = []
    c = c0
    while c < c1:
        out.append((c, min(c + step, c1)))
        c = out[-1][1]
    return out


def _runs(blocks):
    """Contiguous runs from a sorted list of block indices."""
    runs = []
    for i in blocks:
        if runs and runs[-1][1] == i:
            runs[-1][1] = i + 1
        else:
            runs.append([i, i + 1])
    return [tuple(r) for r in runs]


def build_nc(codes, tile_idx, n_bt, mmdt_name="float16"):
    import concourse.bass as bass
    import concourse.mybir as mybir
    import concourse.tile as tile
    from concourse import bacc
    from concourse.masks import make_identity
    from concourse.tile_rust import add_dep_helper

    dt = mybir.dt
    f32 = dt.float32
    mmdt = getattr(dt, mmdt_name)
    Exp = mybir.ActivationFunctionType.Exp
    mult = mybir.AluOpType.mult

    # Per key-block: which q-blocks participate.
    active = {j: [i for i in range(NB) if codes[i, j] != SKIP] for j in range(NB)}
    for i in range(NB):
        assert any(codes[i, j] != SKIP for j in range(NB)), (
            "query block with all key blocks masked should be impossible"
        )
    # PV PSUM accumulation start/stop must be managed per 512-column PSUM
    # bank (4 q-blocks): first/last key-block writing each bank.
    NBANK = 4
    bank_first = {}
    bank_last = {}
    for bank in range(NBANK):
        js = [
            j
            for j in range(NB)
            if any(codes[i, j] != SKIP for i in range(bank * 4, bank * 4 + 4))
        ]
        bank_first[bank] = js[0]
        bank_last[bank] = js[-1]

    nc = bacc.Bacc("TRN2", target_bir_lowering=False, debug=False, num_devices=NCORES)
    # Q and K arrive pre-transposed ([head, d, seq]) and V pre-arranged in
    # its SBUF layout ([128, nb*65] with the ones column baked in), all
    # pre-cast to the matmul dtype -- every load is one contiguous DMA.
    qt_d = nc.dram_tensor("qt", [HPC, D, S], mmdt, kind="ExternalInput").ap()
    kt_d = nc.dram_tensor("kt", [HPC, D, S], mmdt, kind="ExternalInput").ap()
    v_d = nc.dram_tensor("v", [HPC, BLK, NB * VW], mmdt, kind="ExternalInput").ap()
    bt_d = nc.dram_tensor("bt", [n_bt, BLK, BLK], mmdt, kind="ExternalInput").ap()
    o_d = nc.dram_tensor("o", [HPC, S, D], mmdt, kind="ExternalOutput").ap()

    with tile.TileContext(nc) as tc, ExitStack() as ctx:
        const = ctx.enter_context(tc.tile_pool(name="const", bufs=1))
        ldpool = ctx.enter_context(tc.tile_pool(name="ld", bufs=HPC))
        tpool = ctx.enter_context(tc.tile_pool(name="tp", bufs=HPC))
        ppool = ctx.enter_context(tc.tile_pool(name="pp", bufs=3))
        otpool = ctx.enter_context(tc.tile_pool(name="ot", bufs=2))
        smpool = ctx.enter_context(tc.tile_pool(name="sm", bufs=3))
        outpool = ctx.enter_context(tc.tile_pool(name="ob", bufs=4))
        # PSUM budget (8 banks x 2KB/partition): sc 3 banks + pv 4 + rt 1
        # (the denominator transpose shares the rt bank at cols 512:520).
        # rt/dnt in their own rings (not the pv ring) so the next head's PV
        # accumulation never serializes behind this head's epilogue drain.
        scpool = ctx.enter_context(tc.tile_pool(name="sc", bufs=3, space="PSUM"))
        pvpool = ctx.enter_context(tc.tile_pool(name="pv", bufs=2, space="PSUM"))
        rtpool = ctx.enter_context(tc.tile_pool(name="rt", bufs=1, space="PSUM"))

        identm = const.tile([BLK, BLK], mmdt, tag="identm")
        make_identity(nc, identm[:])

        # PE-order bookkeeping: weight reloads cost ~330ns per stationary
        # swap, so same-weight matmuls must run adjacently. We collect the
        # PE instructions and chain them (sync=False deps) in a software-
        # pipelined order: QK_j+1 before PV_j, epilogue transposes of head h
        # slotted early into head h+1's stream.
        # ---- input loads for all heads, hoisted ahead of the compute so no
        # load DMA trigger ever sits behind a blocking compute instruction on
        # its engine queue. The DMA hardware round-robins active transfers,
        # so a transfer's latency scales with how many are in flight: head
        # 0's loads are split fine (k block 0 and the first q piece finish
        # fast, letting the PE start ~4us earlier) and later heads' loads are
        # gated on head 0's first matmul (via add_dep below) so they don't
        # steal preamble bandwidth.
        qts, kts, vnos, load_insts = [], [], [], []
        for h in range(HPC):
            lds = []
            qt = tpool.tile([D, S], mmdt, tag="qt", name=f"qt{h}")
            kt = tpool.tile([D, S], mmdt, tag="kt", name=f"kt{h}")
            lds.append(nc.scalar.dma_start(out=kt[:, 0:BLK], in_=kt_d[h][:, 0:BLK]))
            if h == 0:
                # small leading pieces so the first QK matmul fires ASAP
                for (c0, c1) in [(0, 256), (256, 512), (512, 1024), (1024, 1536), (1536, 2048)]:
                    lds.append(nc.sync.dma_start(
                        out=qt[:, c0:c1], in_=qt_d[h][:, c0:c1]
                    ))
            else:
                lds.append(nc.sync.dma_start(out=qt[:], in_=qt_d[h]))
            lds.append(nc.scalar.dma_start(out=kt[:, BLK:], in_=kt_d[h][:, BLK:]))
            vno = ldpool.tile([BLK, NB * VW], mmdt, tag="vn", name=f"vn{h}")
            lds.append(nc.gpsimd.dma_start(out=vno[:], in_=v_d[h]))
            qts.append(qt)
            kts.append(kt)
            vnos.append(vno)
            load_insts.append(lds)
            if h == 0:
                # mask keep-tiles right after the head-0 loads on the sync
                # queue (before the gated later-head triggers, which block it)
                bts = []
                for u in range(n_bt):
                    t = const.tile([BLK, BLK], mmdt, tag=f"bt{u}", name=f"bt_sb{u}")
                    nc.sync.dma_start(out=t[:], in_=bt_d[u])
                    bts.append(t)

        qk_h = []
        pv_h = []
        epi_h = []
        epi0_h = []
        for h in range(HPC):
            qk_groups = []
            pv_groups = []
            epi_insts = []
            epi0_insts = []
            qt, kt, vno = qts[h], kts[h], vnos[h]
            vno3 = vno[:].rearrange("p (n c) -> p n c", c=VW)

            # ---- main loop over key blocks ----
            pvh = [
                pvpool.tile([VW, 1024], f32, tag="pv", name=f"pv{h}_{i}")
                for i in range(2)
            ]
            for j in range(NB):
                blocks = active[j]
                if not blocks:
                    continue
                pT = ppool.tile([BLK, S], mmdt, tag="pT")
                qk_g = []
                for (r0, r1) in _runs(blocks):
                    for (c0, c1) in _ceil_pieces(r0 * BLK, r1 * BLK, 512):
                        w = c1 - c0
                        sc = scpool.tile([BLK, w], f32, tag="sc")
                        qk_g.append(nc.tensor.matmul(
                            sc[:, 0:w],
                            lhsT=kt[:, j * BLK:(j + 1) * BLK],
                            rhs=qt[:, c0:c1],
                            start=True,
                            stop=True,
                        ))
                        nc.scalar.activation(pT[:, c0:c1], sc[:, 0:w], Exp, scale=SCALE)
                qk_groups.append(qk_g)
                pv_g = []
                # mixed blocks: zero the masked probabilities (gpsimd is idle)
                for i in blocks:
                    if codes[i, j] == BIAS:
                        sl = pT[:, i * BLK:(i + 1) * BLK]
                        nc.gpsimd.tensor_tensor(sl, sl, bts[tile_idx[i, j]][:], mult)
                # PV accumulation: start/stop flags at PSUM-bank granularity
                bank_order = sorted(
                    range(NBANK),
                    key=lambda b: any(
                        codes[i, j] == BIAS
                        for i in range(b * 4, b * 4 + 4)
                        if i in blocks
                    ),
                )
                for bank in bank_order:
                    bi = [i for i in blocks if bank * 4 <= i < bank * 4 + 4]
                    if not bi:
                        continue
                    half = bank // 2
                    toff = half * 1024  # tile-relative offset of this half
                    is_last = j == bank_last[bank]
                    if j == bank_first[bank]:
                        # first write: one full-bank matmul so every column
                        # starts with start=True; zero any inactive columns
                        # of pT first (no-op for causal/empty masks).
                        for i in range(bank * 4, bank * 4 + 4):
                            if i not in bi:
                                nc.gpsimd.memset(
                                    pT[:, i * BLK:(i + 1) * BLK], 0.0
                                )
                        g0, g1 = bank * 4 * BLK, (bank + 1) * 4 * BLK
                        pv_g.append(nc.tensor.matmul(
                            pvh[half][:, g0 - toff:g1 - toff],
                            lhsT=vno3[:, j, :],
                            rhs=pT[:, g0:g1],
                            start=True,
                            stop=is_last,
                        ))
                    else:
                        runs = _runs(bi)
                        for ri, (r0, r1) in enumerate(runs):
                            pv_g.append(nc.tensor.matmul(
                                pvh[half][:, r0 * BLK - toff:r1 * BLK - toff],
                                lhsT=vno3[:, j, :],
                                rhs=pT[:, r0 * BLK:r1 * BLK],
                                start=False,
                                stop=is_last and ri == len(runs) - 1,
                            ))
                pv_groups.append(pv_g)

            # ---- epilogue: normalize and write out ----
            # Half 0 of the PV accumulator is complete once its last key
            # block is done (j=bank_last[1]), so its copy + retranspose are
            # scheduled mid-k-loop; half 1 and the denominators drain into
            # the next head's stream. All staging is fp16: the PE transposes
            # run at 1 cyc/col (vs 2 for f32) and SBUF/PSUM traffic halves.
            ot = otpool.tile([VW, S], mmdt, tag="ot")
            # Each half: drain the PSUM accumulator to SBUF (512-wide pieces
            # so the retransposes can chase the drain; GpSimd can't read PSUM
            # -- neuronxcc rejects that lowering), retranspose the 8 blocks
            # into rt (cols 0:512) and the denominator row into rt cols
            # 512:520 (sharing the PSUM tile keeps the pvpool ring simple),
            # then normalize + DMA out. Half 0 completes mid-k-loop (its last
            # key block is j=7) so its drain overlaps the tail of the loop;
            # half 1 drains into the next head's stream. Per-half emission
            # order matters: the DVE executes its queue in order, so half 0's
            # reciprocal must not sit behind half 1's (j=15-gated) drain.
            for half in range(2):
                for piece in range(2):
                    nc.vector.tensor_copy(
                        ot[:, half * 1024 + piece * 512:half * 1024 + piece * 512 + 512],
                        pvh[half][:, piece * 512:piece * 512 + 512],
                    )
                rt = rtpool.tile([BLK, 520], mmdt, tag="rt", name=f"rt{h}_{half}")
                dst_list = epi0_insts if half == 0 else epi_insts
                for u in range(8):
                    i = half * 8 + u
                    dst_list.append(nc.tensor.transpose(
                        rt[:, u * D:(u + 1) * D],
                        ot[0:D, i * BLK:(i + 1) * BLK],
                        identm[0:D, 0:D],
                    ))
                # gather denominators [1, 1024] -> [8, 128] -> [128, 8]
                dq = smpool.tile([8, BLK], mmdt, tag="dq")
                nc.sync.dma_start(
                    out=dq[:], in_=ot[D:VW, half * 1024:(half + 1) * 1024]
                )
                dst_list.append(nc.tensor.transpose(
                    rt[:, 512:520], dq[:], identm[0:8, 0:8]
                ))
                rcp = smpool.tile([BLK, 8], f32, tag="rcp")
                nc.vector.reciprocal(rcp[:], rt[:, 512:520])
                osb = outpool.tile([BLK, 512], mmdt, tag="ob")
                nc.vector.tensor_tensor(
                    osb[:].rearrange("p (u d) -> p u d", d=D),
                    rt[:, 0:512].rearrange("p (u d) -> p u d", d=D),
                    rcp[:]
                    .rearrange("p (u o) -> p u o", o=1)
                    .broadcast_to([BLK, 8, D]),
                    mult,
                )
                nc.sync.dma_start(
                    out=o_d[h].rearrange("(n p) d -> p n d", p=BLK)[
                        :, half * 8:(half + 1) * 8, :
                    ],
                    in_=osb[:].rearrange("p (u d) -> p u d", d=D),
                )
            qk_h.append(qk_groups)
            pv_h.append(pv_groups)
            epi_h.append(epi_insts)
            epi0_h.append(epi0_insts)

        # Later heads' input loads wait for head 0's first matmul: the DMA
        # engines round-robin active transfers, so an ungated 1.8MB preload
        # burst would delay the first K/Q tiles (and the first matmul) by
        # several microseconds.
        first_mm = qk_h[0][0][0]
        for h in range(1, HPC):
            for ld in load_insts[h]:
                add_dep_helper(ld.ins, first_mm.ins, sync=True, reason="preload gate")

        # Build the PE ordering chain.
        chain = []
        safe0 = max(bank_last[0], bank_last[1]) + 2
        for h in range(HPC):
            qk = qk_h[h]
            pv = pv_h[h]
            assert len(qk) == len(pv)
            placed0 = False
            if qk:
                chain += qk[0]
            for idx in range(1, len(qk)):
                chain += qk[idx]
                if idx == 2 and h > 0:
                    # prev head's epilogue transposes; rt/dnt live in their
                    # own PSUM rings so nothing downstream waits on them
                    chain += epi_h[h - 1]
                if idx == safe0 and not placed0:
                    chain += epi0_h[h]
                    placed0 = True
                chain += pv[idx - 1]
            if pv:
                chain += pv[-1]
            if not placed0:
                chain += epi0_h[h]
        chain += epi_h[HPC - 1]
        for a, b in zip(chain, chain[1:]):
            add_dep_helper(b.ins, a.ins, sync=False, reason="pe weight-group order")
    nc.compile()
    return nc


MM_DT = __import__("os").environ.get("ATTN_MM_DT", "float16")


def _get_program(mask):
    codes, tile_idx, bt = _plan_from_mask(mask)
    key = (codes.tobytes(), tile_idx.tobytes(), bt.tobytes(), MM_DT)
    if key not in _cache:
        _cache[key] = (build_nc(codes, tile_idx, bt.shape[0], MM_DT), bt)
    return _cache[key]


LAST_RESULTS = None  # BassKernelResults of the most recent run (for profiling)


def kernel(q, k, v, mask):
    global LAST_RESULTS
    from concourse.bass_utils import run_bass_kernel_spmd
    import ml_dtypes

    npdt = {"float16": np.float16, "bfloat16": ml_dtypes.bfloat16}[MM_DT]
    nc, bt = _get_program(mask)
    qf = np.asarray(q, np.float32).reshape(BH, S, D)
    kf = np.asarray(k, np.float32).reshape(BH, S, D)
    vf = np.asarray(v, np.float32).reshape(BH, S, D)
    # V in its SBUF layout: [128, nb, 65] per head, ones column baked in.
    vr = vf.reshape(BH, NB, BLK, D).transpose(0, 2, 1, 3)  # [BH, 128, NB, D]
    vno = np.concatenate(
        [vr, np.ones((BH, BLK, NB, 1), np.float32)], axis=3
    ).reshape(BH, BLK, NB * VW).astype(npdt)
    bt16 = bt.astype(npdt)
    in_maps = []
    for c in range(NCORES):
        sl = slice(c * HPC, (c + 1) * HPC)
        in_maps.append({
            # per-shard layout: Q/K shipped [head, d, seq], pre-cast
            "qt": np.ascontiguousarray(qf[sl].transpose(0, 2, 1)).astype(npdt),
            "kt": np.ascontiguousarray(kf[sl].transpose(0, 2, 1)).astype(npdt),
            "v": vno[sl],
            "bt": bt16,
        })
    res = run_bass_kernel_spmd(nc, in_maps, list(range(NCORES)))
    LAST_RESULTS = res
    out = np.concatenate(
        [np.asarray(res.results[c]["o"]) for c in range(NCORES)], axis=0
    )
    return out.reshape(B, H, S, D).astype(np.float32)
